# revision 1
# baseline (speedup 1.0000x reference)
"""Trainium2 Bass kernel for the low-rank linear operator.

Math: the reference collapses algebraically. With y = linspace(-1,1,H),
x = linspace(-1,1,W), dx = 2/(W-1):

  Vy[b,i] = sum_{h,w} v[b,i,h,w] * y_h
  Vx[b,i] = sum_{h,w} v[b,i,h,w] * x_w
  inner[b,r] = dx * sum_i (Vy[b,i]*psi[r,i,0] + Vx[b,i]*psi[r,i,1])
  A[b,o] = sum_r inner[b,r]*phi[o,r,0];  Bc[b,o] = sum_r inner[b,r]*phi[o,r,1]
  u[b,o,h,w] = A[b,o]*y_h + Bc[b,o]*x_w

Sharding: data-parallel over batch, 2 batches per core, 8 cores, no
collectives.

Layout: two h-rows per partition (p = h//2, hh = h%2 on the free axis) so
every DMA descriptor moves 2KB contiguous (the HW profile showed 1KB
descriptors made DMA packet-rate, not bytes, the bottleneck). Since y is
affine (y[2p+hh] = y[2p] + hh*dy), PE matmuls against a [y_even, 1]
stationary still recover the h-weighted sums, with an hh==1 correction
folded into the second reduction pass. ACT/DVE drain psum; a DRAM bounce
re-partitions per-channel rows to [128(2i+p), (hh w)]; full-width DVE
mult+reduce passes produce the (Vy-part, Vx) vectors; tiny PE matmuls give
inner -> (A,B) broadcast into per-partition scale/bias; DVE/ACT/Pool
generate u tiles as x_w*B + y_even*A (+ A*dy on the hh=1 half).
"""

import sys

try:
    import concourse.bass as bass  # noqa: F401
except ImportError:
    for _p in ("/opt/trn_rl_repo", "/root/.axon_site/_ro/trn_rl_repo"):
        if _p not in sys.path:
            sys.path.insert(0, _p)

import numpy as np

import concourse.bacc as bacc
import concourse.bass as bass
import concourse.mybir as mybir
import concourse.tile as tile
from concourse.bass_utils import run_bass_kernel_spmd

F32 = mybir.dt.float32
MULT = mybir.AluOpType.mult
ADD = mybir.AluOpType.add

B, CI, CO, R, H, W = 16, 64, 64, 64, 256, 256
N_CORES = 8
BPC = B // N_CORES  # batches per core
HP = H // 2         # h-pairs per partition dim

# generation-engine rotation
_GEN_ENGINES = ("dve", "act", "dve", "act", "pool", "dve", "act", "pool")


def build_nc():
    nc = bacc.Bacc("TRN2", target_bir_lowering=False, debug=False)

    v = nc.dram_tensor("v", [BPC, CI, H, W], F32, kind="ExternalInput")
    psi2y = nc.dram_tensor("psi2y", [2 * CI, R], F32, kind="ExternalInput")
    psi2x = nc.dram_tensor("psi2x", [2 * CI, R], F32, kind="ExternalInput")
    phicat = nc.dram_tensor("phicat", [R, 2 * CO], F32, kind="ExternalInput")
    wty = nc.dram_tensor("wty", [2 * CI, 2 * W], F32, kind="ExternalInput")
    wtx = nc.dram_tensor("wtx", [2 * CI, 2 * W], F32, kind="ExternalInput")
    y2e = nc.dram_tensor("y2e", [HP, 2], F32, kind="ExternalInput")
    xrep = nc.dram_tensor("xrep", [128, W], F32, kind="ExternalInput")
    ybc = nc.dram_tensor("ybc", [1, 384], F32, kind="ExternalInput")
    ident1 = nc.dram_tensor("ident1", [1, 1], F32, kind="ExternalInput")
    u = nc.dram_tensor("u", [BPC, CO, H, W], F32, kind="ExternalOutput")

    IBLK = 8          # channels per input DMA
    NBLK = CI // IBLK
    OBLK = 4          # output channels per output DMA
    NOBLK = CO // OBLK

    with tile.TileContext(nc) as tc:
        with (
            tc.tile_pool(name="consts", bufs=1) as consts,
            tc.tile_pool(name="inp", bufs=3) as in_pool,
            tc.tile_pool(name="outp", bufs=4) as out_pool,
            tc.tile_pool(name="scr", bufs=3) as scratch,
            tc.tile_pool(name="bc", bufs=6) as bc_pool,
            tc.tile_pool(name="psumP", bufs=5, space="PSUM") as psum_p,
            tc.tile_pool(name="psumT", bufs=1, space="PSUM") as psum_t,
            tc.tile_pool(name="psumBC", bufs=2, space="PSUM") as psum_bc,
            tc.tile_pool(name="dram", bufs=2, space="DRAM") as dram_pool,
        ):
            sb_psi2y = consts.tile([2 * CI, R], F32)
            nc.scalar.dma_start(sb_psi2y[:], psi2y[:])
            sb_psi2x = consts.tile([2 * CI, R], F32)
            nc.scalar.dma_start(sb_psi2x[:], psi2x[:])
            sb_phicat = consts.tile([R, 2 * CO], F32)
            nc.scalar.dma_start(sb_phicat[:], phicat[:])
            sb_wty = consts.tile([2 * CI, 2 * W], F32)
            nc.scalar.dma_start(sb_wty[:], wty[:])
            sb_wtx = consts.tile([2 * CI, 2 * W], F32)
            nc.scalar.dma_start(sb_wtx[:], wtx[:])
            sb_y2e = consts.tile([HP, 2], F32)
            nc.scalar.dma_start(sb_y2e[:], y2e[:])
            sb_xrep = consts.tile([128, W], F32)
            nc.scalar.dma_start(sb_xrep[:], xrep[:])
            sb_ybc = consts.tile([1, 384], F32)
            nc.scalar.dma_start(sb_ybc[:], ybc[:])
            sb_id1 = consts.tile([1, 1], F32)
            nc.scalar.dma_start(sb_id1[:], ident1[:])

            # reduction vectors: partition 2i   -> y-part (needs pair-sum)
            #                    partition 2i+1 -> correction / Vx
            gcaty = consts.tile([2 * CI, BPC], F32)
            gcatx = consts.tile([2 * CI, BPC], F32)

            def phase_a(b, interleave=None):
                """Reduce v[b] -> gcaty/gcatx[:, b]."""
                dscr = dram_pool.tile([CI, 2, 2 * W], F32, tag="dscr")
                drain = 0
                inter = interleave() if interleave is not None else None
                for blk in range(NBLK):
                    if inter is not None:
                        next(inter, None)
                        if blk >= NBLK // 2:
                            next(inter, None)
                    i0 = blk * IBLK
                    t = in_pool.tile([128, IBLK, 2, W], F32, tag="in")
                    nc.sync.dma_start(
                        t[:],
                        v[b, i0 : i0 + IBLK, :, :].rearrange(
                            "i (p hh) w -> p i hh w", p=HP
                        ),
                    )
                    pj = []
                    for ii in range(IBLK):
                        p = psum_p.tile([2, 2, W], F32, tag="P")
                        pj.append(p)
                        nc.tensor.matmul(
                            p[:], lhsT=sb_y2e[:], rhs=t[:, ii, :, :],
                            start=True, stop=True,
                        )
                    s_blk = scratch.tile([2, IBLK, 2 * W], F32, tag="sblk")
                    for ii in range(IBLK):
                        dst = s_blk[:, ii, :]
                        src = pj[ii][:].rearrange("c hh w -> c (hh w)")
                        if drain % 2 == 0:
                            nc.scalar.copy(dst, src)
                        else:
                            nc.vector.tensor_copy(dst, src)
                        drain += 1
                    nc.scalar.dma_start(
                        dscr[i0 : i0 + IBLK, :, :].rearrange("i p f -> p i f"),
                        s_blk[:],
                    )
                # re-partition on readback: dscr[i, p, f] -> s2[2i+p, f]
                s2 = scratch.tile([2 * CI, 2 * W], F32, tag="s2")
                nc.scalar.dma_start(s2[:], dscr[:].rearrange("i p f -> (i p) f"))
                sc2 = scratch.tile([2 * CI, 2 * W], F32, tag="sc2")
                nc.vector.tensor_tensor(out=sc2[:], in0=s2[:], in1=sb_wty[:], op=MULT)
                nc.vector.tensor_reduce(
                    out=gcaty[:, b : b + 1], in_=sc2[:],
                    axis=mybir.AxisListType.X, op=ADD,
                )
                sc3 = scratch.tile([2 * CI, 2 * W], F32, tag="sc2")
                nc.vector.tensor_tensor(out=sc3[:], in0=s2[:], in1=sb_wtx[:], op=MULT)
                nc.vector.tensor_reduce(
                    out=gcatx[:, b : b + 1], in_=sc3[:],
                    axis=mybir.AxisListType.X, op=ADD,
                )

            def tiny(b):
                """gcaty/x[:, b] -> per-partition scale/bias SBUF tiles."""
                inner_ps = psum_t.tile([1, R], F32, tag="tiny")
                nc.tensor.matmul(
                    inner_ps[:], lhsT=gcaty[:, b : b + 1], rhs=sb_psi2y[:],
                    start=True, stop=False,
                )
                nc.tensor.matmul(
                    inner_ps[:], lhsT=gcatx[:, b : b + 1], rhs=sb_psi2x[:],
                    start=False, stop=True,
                )
                sb_inner = scratch.tile([1, R], F32, tag="ti1")
                nc.vector.tensor_copy(sb_inner[:], inner_ps[:])

                innert_ps = psum_t.tile([R, 1], F32, tag="tiny")
                nc.tensor.transpose(innert_ps[:], sb_inner[:], sb_id1[:])
                sb_innert = scratch.tile([R, 1], F32, tag="ti2")
                nc.vector.tensor_copy(sb_innert[:], innert_ps[:])

                ab_ps = psum_t.tile([1, 2 * CO], F32, tag="tiny")
                nc.tensor.matmul(
                    ab_ps[:], lhsT=sb_innert[:], rhs=sb_phicat[:],
                    start=True, stop=True,
                )
                sb_ab = scratch.tile([1, 2 * CO], F32, tag="ti3")
                nc.vector.tensor_copy(sb_ab[:], ab_ps[:])

                outs = []
                for k in range(3):  # bias_even (A*y_even), bias_odd (A*y_odd), scale (B)
                    ps = psum_bc.tile([128, 2 * CO], F32, tag="bc")
                    nc.tensor.matmul(
                        ps[:],
                        lhsT=sb_ybc[0:1, 128 * k : 128 * (k + 1)],
                        rhs=sb_ab[:],
                        start=True,
                        stop=True,
                    )
                    sb = bc_pool.tile([128, 2 * CO], F32, tag="bcs")
                    nc.vector.tensor_copy(sb[:], ps[:])
                    outs.append(sb)
                return outs  # [bias_even, bias_odd, scale]

            def _phase_b_gen(b, bias_e, bias_o, scale):
                eng = 0
                for oc in range(NOBLK):
                    yield
                    ot = out_pool.tile([128, OBLK, 2, W], F32, tag="out")
                    for ol in range(OBLK):
                        o = oc * OBLK + ol
                        sc_ap = scale[:, 2 * o + 1 : 2 * o + 2]
                        for hh in range(2):
                            bias_ap = (bias_e if hh == 0 else bias_o)[:, 2 * o : 2 * o + 1]
                            dst = ot[:, ol, hh, :]
                            which = _GEN_ENGINES[eng % len(_GEN_ENGINES)]
                            eng += 1
                            if which == "dve":
                                nc.vector.tensor_scalar(
                                    out=dst, in0=sb_xrep[:], scalar1=sc_ap,
                                    scalar2=bias_ap, op0=MULT, op1=ADD,
                                )
                            elif which == "pool":
                                nc.gpsimd.tensor_scalar(
                                    out=dst, in0=sb_xrep[:], scalar1=sc_ap,
                                    scalar2=bias_ap, op0=MULT, op1=ADD,
                                )
                            else:
                                nc.scalar.activation(
                                    dst, sb_xrep[:],
                                    mybir.ActivationFunctionType.Identity,
                                    bias=bias_ap, scale=sc_ap,
                                )
                    nc.scalar.dma_start(
                        u[b, oc * OBLK : (oc + 1) * OBLK, :, :].rearrange(
                            "o (p hh) w -> p o hh w", p=128
                        ),
                        ot[:],
                    )

            phase_a(0)
            sb0 = tiny(0)
            b0_gen = _phase_b_gen(0, *sb0)
            phase_a(1, interleave=lambda: b0_gen)
            for _ in b0_gen:
                pass
            sb1 = tiny(1)
            for _ in _phase_b_gen(1, *sb1):
                pass

    nc.compile()
    return nc


def make_in_maps(v, psi, phi):
    y = np.linspace(-1.0, 1.0, H, dtype=np.float32)
    x = np.linspace(-1.0, 1.0, W, dtype=np.float32)
    dx = np.float32(2.0 / (W - 1))
    dy = np.float32(2.0 / (H - 1))
    ones = np.ones(128, dtype=np.float32)

    # psi packs: inner = sum_q gy[q]*psi2y[q, r] + gx[q]*psi2x[q, r]
    # gy[2i] + gy[2i+1] = Vy[i]; gx[2i+1] = Vx[i], gx[2i] = 0
    psi2y = np.empty((2 * CI, R), np.float32)
    psi2y[0::2, :] = psi[:, :, 0].T * dx
    psi2y[1::2, :] = psi[:, :, 0].T * dx
    psi2x = np.zeros((2 * CI, R), np.float32)
    psi2x[1::2, :] = psi[:, :, 1].T * dx

    phicat = np.stack([phi[:, :, 0].T, phi[:, :, 1].T], axis=2).reshape(R, 2 * CO)

    # reduction weights over s2[2i+p, (hh w)]:
    #  row 2i   = y_even-weighted sums -> Vy part, weight 1
    #  row 2i+1 = per-hh colsums -> Vy correction dy*[hh==1]; Vx weight x_w
    wty = np.zeros((2 * CI, 2 * W), np.float32)
    wty[0::2, :] = 1.0
    wty[1::2, W:] = dy
    wtx = np.zeros((2 * CI, 2 * W), np.float32)
    wtx[1::2, 0:W] = x
    wtx[1::2, W:] = x

    shards = np.ascontiguousarray(v.reshape(N_CORES, BPC, CI, H, W))
    common = {
        "psi2y": psi2y,
        "psi2x": psi2x,
        "phicat": np.ascontiguousarray(phicat),
        "wty": wty,
        "wtx": wtx,
        "y2e": np.stack([y[0::2], ones], axis=1).astype(np.float32),
        "xrep": np.broadcast_to(x, (128, W)).copy(),
        "ybc": np.concatenate([y[0::2], y[1::2], ones])[None, :].astype(np.float32),
        "ident1": np.ones((1, 1), dtype=np.float32),
    }
    return [{"v": shards[i], **common} for i in range(N_CORES)]


_NC_CACHE = None


def kernel(v, psi, phi):
    global _NC_CACHE
    if _NC_CACHE is None:
        _NC_CACHE = build_nc()
    nc = _NC_CACHE
    in_maps = make_in_maps(
        np.ascontiguousarray(v, dtype=np.float32),
        np.asarray(psi, dtype=np.float32),
        np.asarray(phi, dtype=np.float32),
    )
    res = run_bass_kernel_spmd(nc, in_maps, core_ids=list(range(N_CORES)))
    return np.concatenate([r["u"] for r in res.results], axis=0)


if __name__ == "__main__":
    build_nc()
    print("build ok")



# revision 23
# speedup vs baseline: 1.3913x; 1.3913x over previous
"""Trainium2 Bass kernel for the low-rank linear operator.

Math: the reference collapses algebraically. With y = linspace(-1,1,H),
x = linspace(-1,1,W), dx = 2/(W-1):

  Vy[b,i] = sum_{h,w} v[b,i,h,w] * y_h
  Vx[b,i] = sum_{h,w} v[b,i,h,w] * x_w
  inner[b,r] = dx * sum_i (Vy[b,i]*psi[r,i,0] + Vx[b,i]*psi[r,i,1])
  A[b,o] = sum_r inner[b,r]*phi[o,r,0];  Bc[b,o] = sum_r inner[b,r]*phi[o,r,1]
  u[b,o,h,w] = A[b,o]*y_h + Bc[b,o]*x_w

Sharding: data-parallel over batch, 2 batches per core, 8 cores, no
collectives.

The kernel is HBM-bandwidth bound (read v + write u). All HBM traffic is
bf16 (host casts in/out; rel-err budget 2e-2 vs ~0.5% incurred), halving
bytes vs f32. Layout: four h-rows per partition, p = 64*(i%2) + h//4, so
every DMA descriptor moves 2KB contiguous even at 2B/elem.

Reduction: PE matmul with a block-diagonal [128,4] stationary reduces the
h-quad partition dim for both channel parities at once -> psum [4, 512]
rows (even-ch y-weighted, even-ch colsum, odd-ch y-weighted, odd-ch
colsum). Since y is affine, y[4q+hh] = y[4q] + hh*dy, the hh correction is
folded into a second full-width DVE mult+reduce pass after an SBUF->SBUF
DMA re-partitions the drained psum rows to [128, (hh w)]. Tiny PE matmuls
give inner -> (A,B) -> per-partition scale/bias; DVE/ACT/Pool generate u
tiles as x_w*B + (A*y) bias directly in bf16.
"""

import sys

try:
    import concourse.bass as bass  # noqa: F401
except ImportError:
    for _p in ("/opt/trn_rl_repo", "/root/.axon_site/_ro/trn_rl_repo"):
        if _p not in sys.path:
            sys.path.insert(0, _p)

import ml_dtypes
import numpy as np

import concourse.bacc as bacc
import concourse.bass as bass
import concourse.mybir as mybir
import concourse.tile as tile
from concourse.bass_utils import run_bass_kernel_spmd

F32 = mybir.dt.float32
BF16 = mybir.dt.bfloat16
MULT = mybir.AluOpType.mult
ADD = mybir.AluOpType.add
BFNP = ml_dtypes.bfloat16

B, CI, CO, R, H, W = 16, 64, 64, 64, 256, 256
N_CORES = 8
BPC = B // N_CORES  # batches per core
NBLK = 8            # input DMA blocks per batch (8 channels each)
NTG = 8             # output DMA blocks per batch (8 channels each)

# generation-engine rotation
_GEN_ENGINES = ("dve", "act", "pool", "dve", "act", "dve", "act", "pool")


def build_nc():
    nc = bacc.Bacc("TRN2", target_bir_lowering=False, debug=False)

    v = nc.dram_tensor("v", [BPC, CI, H, W], BF16, kind="ExternalInput")
    y4cat = nc.dram_tensor("y4cat", [128, 4], BF16, kind="ExternalInput")
    wty = nc.dram_tensor("wty", [128, 4, W], BF16, kind="ExternalInput")
    wtx = nc.dram_tensor("wtx", [128, 4, W], BF16, kind="ExternalInput")
    psiy_p = nc.dram_tensor("psiy_p", [128, R], F32, kind="ExternalInput")
    psix_p = nc.dram_tensor("psix_p", [128, R], F32, kind="ExternalInput")
    phip0 = nc.dram_tensor("phip0", [R, CO], F32, kind="ExternalInput")
    phip1 = nc.dram_tensor("phip1", [R, CO], F32, kind="ExternalInput")
    erow0 = nc.dram_tensor("erow0", [1, 128], F32, kind="ExternalInput")
    erow1 = nc.dram_tensor("erow1", [1, 128], F32, kind="ExternalInput")
    ycol4 = nc.dram_tensor("ycol4", [128, 4], F32, kind="ExternalInput")
    xrep = nc.dram_tensor("xrep", [128, W], F32, kind="ExternalInput")
    ident1 = nc.dram_tensor("ident1", [1, 1], F32, kind="ExternalInput")
    u = nc.dram_tensor("u", [BPC, CO, H, W], BF16, kind="ExternalOutput")

    with tile.TileContext(nc) as tc:
        with (
            tc.tile_pool(name="consts", bufs=1) as consts,
            tc.tile_pool(name="inp", bufs=1) as in_pool,
            tc.tile_pool(name="outp", bufs=3) as out_pool,
            tc.tile_pool(name="scr", bufs=1) as scratch,
            tc.tile_pool(name="sblkp", bufs=2) as sblk_pool,
            tc.tile_pool(name="s2p", bufs=1) as s2_pool,
            tc.tile_pool(name="bc", bufs=2) as bc_pool,
            tc.tile_pool(name="psumP", bufs=5, space="PSUM") as psum_p,
            tc.tile_pool(name="psumT", bufs=1, space="PSUM") as psum_t,
            tc.tile_pool(name="psumBC", bufs=1, space="PSUM") as psum_bc,
            tc.tile_pool(name="dram", bufs=2, space="DRAM") as dram_pool,
        ):
            sb_y4cat = consts.tile([128, 4], BF16)
            nc.scalar.dma_start(sb_y4cat[:], y4cat[:])
            sb_wty = consts.tile([128, 4, W], BF16)
            nc.scalar.dma_start(sb_wty[:], wty[:])
            sb_wtx = consts.tile([128, 4, W], BF16)
            nc.scalar.dma_start(sb_wtx[:], wtx[:])
            sb_psiy = consts.tile([128, R], F32)
            nc.scalar.dma_start(sb_psiy[:], psiy_p[:])
            sb_psix = consts.tile([128, R], F32)
            nc.scalar.dma_start(sb_psix[:], psix_p[:])
            sb_phip0 = consts.tile([R, CO], F32)
            nc.scalar.dma_start(sb_phip0[:], phip0[:])
            sb_phip1 = consts.tile([R, CO], F32)
            nc.scalar.dma_start(sb_phip1[:], phip1[:])
            sb_erow0 = consts.tile([1, 128], F32)
            nc.scalar.dma_start(sb_erow0[:], erow0[:])
            sb_erow1 = consts.tile([1, 128], F32)
            nc.scalar.dma_start(sb_erow1[:], erow1[:])
            sb_ycol4 = consts.tile([128, 4], F32)
            nc.scalar.dma_start(sb_ycol4[:], ycol4[:])
            sb_xrep = consts.tile([128, W], F32)
            nc.scalar.dma_start(sb_xrep[:], xrep[:])
            sb_id1 = consts.tile([1, 1], F32)
            nc.scalar.dma_start(sb_id1[:], ident1[:])

            # prefetch ALL input tiles (both batches) on the sync queue so
            # the read stream never stalls behind anything else
            in_tiles = []
            for b in range(BPC):
                row = []
                for blk in range(NBLK):
                    t = in_pool.tile(
                        [128, 4, 4, W], BF16, tag=f"in{(b * NBLK + blk) % 12}"
                    )
                    nc.sync.dma_start(
                        t[:],
                        v[b, blk * 8 : blk * 8 + 8, :, :].rearrange(
                            "(i2 ic) (q hh) w -> (ic q) i2 hh w", i2=4, ic=2, q=64, hh=4
                        ),
                    )
                    row.append(t)
                in_tiles.append(row)

            def reduce_phase(b, interleave=None):
                """v[b] tiles -> s2[b]: [128=(ihi r), (hh w)] partial sums."""
                s2 = s2_pool.tile([128, 4, W], BF16, tag=f"s2{b}")
                dscr = dram_pool.tile([32, 4, 2, 512], BF16, tag="dscr")
                drain = 0
                inter = interleave() if interleave is not None else None
                for blk in range(NBLK):
                    if inter is not None:
                        next(inter, None)
                    t = in_tiles[b][blk]
                    s_blk = sblk_pool.tile([4, 4, 2, 512], BF16, tag="sblk")
                    for i2 in range(4):
                        for s in range(2):
                            p = psum_p.tile([4, 512], F32, tag="P")
                            nc.tensor.matmul(
                                p[:],
                                lhsT=sb_y4cat[:],
                                rhs=t[:, i2, 2 * s : 2 * s + 2, :],
                                start=True,
                                stop=True,
                            )
                            dst = s_blk[:, i2, s, :]
                            if drain % 2 == 0:
                                nc.vector.tensor_copy(dst, p[:])
                            else:
                                nc.scalar.copy(dst, p[:])
                            drain += 1
                    # bounce out: dscr[4*blk + i2, r, s, f] = s_blk[r, i2, s, f]
                    nc.scalar.dma_start(
                        dscr[4 * blk : 4 * blk + 4].rearrange(
                            "i2 r s f -> r i2 s f"
                        ),
                        s_blk[:],
                    )
                # re-partition on readback: dscr[ihi, r, s, f] -> s2[4*ihi + r, :]
                nc.scalar.dma_start(
                    s2[:].rearrange("p hh w -> p (hh w)"),
                    dscr[:].rearrange("ihi r s f -> (ihi r) (s f)"),
                )
                # full-width weighted reductions -> gy, gx [128, 1]
                prod = scratch.tile([128, 4, W], F32, tag="prod")
                gy = scratch.tile([128, 1], F32, tag=f"gy{b}")
                gx = scratch.tile([128, 1], F32, tag=f"gx{b}")
                nc.vector.tensor_tensor(out=prod[:], in0=s2[:], in1=sb_wty[:], op=MULT)
                nc.vector.tensor_reduce(
                    out=gy[:], in_=prod[:], axis=mybir.AxisListType.XY, op=ADD
                )
                prod2 = scratch.tile([128, 4, W], F32, tag="prod")
                nc.vector.tensor_tensor(out=prod2[:], in0=s2[:], in1=sb_wtx[:], op=MULT)
                nc.vector.tensor_reduce(
                    out=gx[:], in_=prod2[:], axis=mybir.AxisListType.XY, op=ADD
                )
                return gy, gx

            def tiny(b, gy, gx):
                """gy/gx -> ABcols [128, 32, 2] (A,B per ch-pair) + biasT [128, 4, 32]."""
                inner_ps = psum_t.tile([1, R], F32, tag="tiny")
                nc.tensor.matmul(
                    inner_ps[:], lhsT=gy[:], rhs=sb_psiy[:], start=True, stop=False
                )
                nc.tensor.matmul(
                    inner_ps[:], lhsT=gx[:], rhs=sb_psix[:], start=False, stop=True
                )
                sb_inner = scratch.tile([1, R], F32, tag="ti1")
                nc.vector.tensor_copy(sb_inner[:], inner_ps[:])

                innert_ps = psum_t.tile([R, 1], F32, tag="tinyT")
                nc.tensor.transpose(innert_ps[:], sb_inner[:], sb_id1[:])
                sb_innert = scratch.tile([R, 1], F32, tag="ti2")
                nc.vector.tensor_copy(sb_innert[:], innert_ps[:])

                ab_sb = []
                for c, phip in ((0, sb_phip0), (1, sb_phip1)):
                    ab_ps = psum_t.tile([1, CO], F32, tag="tiny")
                    nc.tensor.matmul(
                        ab_ps[:], lhsT=sb_innert[:], rhs=phip[:], start=True, stop=True
                    )
                    sb = scratch.tile([1, CO], F32, tag=f"ti3{c}")
                    nc.vector.tensor_copy(sb[:], ab_ps[:])
                    ab_sb.append(sb)

                # ABcols[p, t, j] = (A,B)[2t + p//64, j]: two rank-1 outer
                # products (indicator-row x ab-row) accumulated on PE
                e_ps = psum_bc.tile([128, 32, 2], F32, tag="bc")
                nc.tensor.matmul(
                    e_ps[:], lhsT=sb_erow0[:], rhs=ab_sb[0][:],
                    start=True, stop=False,
                )
                nc.tensor.matmul(
                    e_ps[:], lhsT=sb_erow1[:], rhs=ab_sb[1][:],
                    start=False, stop=True,
                )
                abcols = bc_pool.tile([128, 32, 2], F32, tag="abcols")
                nc.vector.tensor_copy(abcols[:], e_ps[:])
                biast = bc_pool.tile([128, 4, 32], F32, tag="biast")
                for hh in range(4):
                    nc.vector.tensor_scalar(
                        out=biast[:, hh, :], in0=abcols[:, :, 0],
                        scalar1=sb_ycol4[:, hh : hh + 1], scalar2=None, op0=MULT,
                    )
                return abcols, biast

            def gen_stream(b, abcols, biast):
                eng = 0
                for tg in range(NTG):
                    yield
                    ot = out_pool.tile([128, 4, 4, W], BF16, tag="out")
                    for tl in range(4):
                        ti = 4 * tg + tl
                        sc_ap = abcols[:, ti, 1:2]
                        for hh in range(4):
                            bias_ap = biast[:, hh, ti : ti + 1]
                            dst = ot[:, tl, hh, :]
                            which = _GEN_ENGINES[eng % len(_GEN_ENGINES)]
                            eng += 1
                            if which == "dve":
                                nc.vector.tensor_scalar(
                                    out=dst, in0=sb_xrep[:], scalar1=sc_ap,
                                    scalar2=bias_ap, op0=MULT, op1=ADD,
                                )
                            elif which == "pool":
                                nc.gpsimd.tensor_scalar(
                                    out=dst, in0=sb_xrep[:], scalar1=sc_ap,
                                    scalar2=bias_ap, op0=MULT, op1=ADD,
                                )
                            else:
                                nc.scalar.activation(
                                    dst, sb_xrep[:],
                                    mybir.ActivationFunctionType.Identity,
                                    bias=bias_ap, scale=sc_ap,
                                )
                    nc.scalar.dma_start(
                        u[b, tg * 8 : tg * 8 + 8, :, :].rearrange(
                            "(tl ic) (q hh) w -> (ic q) tl hh w", tl=4, ic=2, q=64, hh=4
                        ),
                        ot[:],
                    )

            gy0, gx0 = reduce_phase(0)
            ab0 = tiny(0, gy0, gx0)
            g0 = gen_stream(0, *ab0)
            gy1, gx1 = reduce_phase(1, interleave=lambda: g0)
            for _ in g0:
                pass
            ab1 = tiny(1, gy1, gx1)
            for _ in gen_stream(1, *ab1):
                pass

    nc.compile()
    return nc


def make_in_maps(v, psi, phi):
    y = np.linspace(-1.0, 1.0, H, dtype=np.float32)
    x = np.linspace(-1.0, 1.0, W, dtype=np.float32)
    dx = np.float32(2.0 / (W - 1))
    dy = np.float32(2.0 / (H - 1))

    q = np.arange(64)
    # stationary for the h-quad reduction: block-diagonal by channel parity
    # cols: [y4*even, 1*even, y4*odd, 1*odd]; partition p = 64*ic + q
    y4cat = np.zeros((128, 4), np.float32)
    y4cat[0:64, 0] = y[4 * q]
    y4cat[0:64, 1] = 1.0
    y4cat[64:128, 2] = y[4 * q]
    y4cat[64:128, 3] = 1.0

    # s2 partition layout: P = 4*ihi + r, r = 2*ic + role, i = 2*ihi + ic
    # role 0 rows hold y4-weighted sums (weight 1); role 1 rows hold per-hh
    # colsums (Vy correction dy*hh; Vx weight x_w)
    P = np.arange(128)
    role = P % 2
    ic = (P % 4) // 2
    i_of_p = 2 * (P // 4) + ic
    wty = np.zeros((128, 4, W), np.float32)
    wtx = np.zeros((128, 4, W), np.float32)
    wty[role == 0, :, :] = 1.0
    wty[role == 1, :, :] = (dy * np.arange(4, dtype=np.float32))[None, :, None]
    wtx[role == 1, :, :] = x[None, None, :]

    # gy/gx -> inner: psi packs indexed by the same P layout
    psiy_p = (dx * psi[:, i_of_p, 0].T).astype(np.float32)
    psix_p = (dx * psi[:, i_of_p, 1].T).astype(np.float32)
    psix_p[role == 0, :] = 0.0

    # inner -> (A,B) interleaved per channel pair: phip_c[r, 2t+j] for o=2t+c
    t_idx = np.arange(32)
    phip0 = np.zeros((R, CO), np.float32)
    phip1 = np.zeros((R, CO), np.float32)
    for j in range(2):
        phip0[:, 2 * t_idx + j] = phi[2 * t_idx, :, j].T
        phip1[:, 2 * t_idx + j] = phi[2 * t_idx + 1, :, j].T

    erow0 = np.zeros((1, 128), np.float32)
    erow0[0, 0:64] = 1.0
    erow1 = np.zeros((1, 128), np.float32)
    erow1[0, 64:128] = 1.0

    ycol4 = np.empty((128, 4), np.float32)
    for hh in range(4):
        ycol4[0:64, hh] = y[4 * q + hh]
        ycol4[64:128, hh] = y[4 * q + hh]

    shards = np.ascontiguousarray(
        v.reshape(N_CORES, BPC, CI, H, W).astype(BFNP)
    )
    common = {
        "y4cat": y4cat.astype(BFNP),
        "wty": wty.astype(BFNP),
        "wtx": wtx.astype(BFNP),
        "psiy_p": psiy_p,
        "psix_p": psix_p,
        "phip0": phip0,
        "phip1": phip1,
        "erow0": erow0,
        "erow1": erow1,
        "ycol4": ycol4,
        "xrep": np.broadcast_to(x, (128, W)).copy(),
        "ident1": np.ones((1, 1), dtype=np.float32),
    }
    return [{"v": shards[i], **common} for i in range(N_CORES)]


_NC_CACHE = None


def kernel(v, psi, phi):
    global _NC_CACHE
    if _NC_CACHE is None:
        _NC_CACHE = build_nc()
    nc = _NC_CACHE
    in_maps = make_in_maps(
        np.ascontiguousarray(v, dtype=np.float32),
        np.asarray(psi, dtype=np.float32),
        np.asarray(phi, dtype=np.float32),
    )
    res = run_bass_kernel_spmd(nc, in_maps, core_ids=list(range(N_CORES)))
    return np.concatenate(
        [r["u"].astype(np.float32) for r in res.results], axis=0
    )


if __name__ == "__main__":
    build_nc()
    print("build ok")


# revision 35
# speedup vs baseline: 1.4553x; 1.0460x over previous
"""Trainium2 Bass kernel for the low-rank linear operator.

Math: the reference collapses algebraically. With y = linspace(-1,1,H),
x = linspace(-1,1,W), dx = 2/(W-1):

  Vy[b,i] = sum_{h,w} v[b,i,h,w] * y_h
  Vx[b,i] = sum_{h,w} v[b,i,h,w] * x_w
  inner[b,r] = dx * sum_i (Vy[b,i]*psi[r,i,0] + Vx[b,i]*psi[r,i,1])
  A[b,o] = sum_r inner[b,r]*phi[o,r,0];  Bc[b,o] = sum_r inner[b,r]*phi[o,r,1]
  u[b,o,h,w] = A[b,o]*y_h + Bc[b,o]*x_w

Sharding: data-parallel over batch, 2 batches per core, 8 cores, no
collectives.

The kernel is HBM-bandwidth bound (read v + write u). All HBM traffic is
bf16 (host casts in/out; rel-err budget 2e-2 vs ~0.5% incurred), halving
bytes vs f32. Layout: four h-rows per partition, p = 64*(i%2) + h//4, so
every DMA descriptor moves 2KB contiguous even at 2B/elem.

Reduction: PE matmul with a block-diagonal [128,4] stationary reduces the
h-quad partition dim for both channel parities at once -> psum [4, 512]
rows (even-ch y-weighted, even-ch colsum, odd-ch y-weighted, odd-ch
colsum). Since y is affine, y[4q+hh] = y[4q] + hh*dy, the hh correction is
folded into a second full-width DVE mult+reduce pass after an SBUF->SBUF
DMA re-partitions the drained psum rows to [128, (hh w)]. Tiny PE matmuls
give inner -> (A,B) -> per-partition scale/bias; DVE/ACT/Pool generate u
tiles as x_w*B + (A*y) bias directly in bf16.
"""

import sys

try:
    import concourse.bass as bass  # noqa: F401
except ImportError:
    for _p in ("/opt/trn_rl_repo", "/root/.axon_site/_ro/trn_rl_repo"):
        if _p not in sys.path:
            sys.path.insert(0, _p)

import ml_dtypes
import numpy as np

import concourse.bacc as bacc
import concourse.bass as bass
import concourse.mybir as mybir
import concourse.tile as tile
from concourse.bass_utils import run_bass_kernel_spmd

F32 = mybir.dt.float32
BF16 = mybir.dt.bfloat16
MULT = mybir.AluOpType.mult
ADD = mybir.AluOpType.add
BFNP = ml_dtypes.bfloat16

B, CI, CO, R, H, W = 16, 64, 64, 64, 256, 256
N_CORES = 8
BPC = B // N_CORES  # batches per core
NBLK = 8            # input DMA blocks per batch (8 channels each)
NTG = 8             # output DMA blocks per batch (8 channels each)

# generation-engine rotation (gpsimd has no drain duty -> give it more)
_GEN_ENGINES = (
    "pool", "dve", "act", "pool", "dve", "act", "pool", "dve",
    "act", "pool", "dve", "act", "pool", "dve", "act", "pool",
)


def build_nc():
    nc = bacc.Bacc("TRN2", target_bir_lowering=False, debug=False)

    v = nc.dram_tensor("v", [BPC, CI, H, W], BF16, kind="ExternalInput")
    y4cat = nc.dram_tensor("y4cat", [128, 4], BF16, kind="ExternalInput")
    wty = nc.dram_tensor("wty", [128, 4, W], BF16, kind="ExternalInput")
    wtx = nc.dram_tensor("wtx", [128, 4, W], BF16, kind="ExternalInput")
    my_cat = nc.dram_tensor("my_cat", [128, 2 * CO], F32, kind="ExternalInput")
    mx_cat = nc.dram_tensor("mx_cat", [128, 2 * CO], F32, kind="ExternalInput")
    erow0 = nc.dram_tensor("erow0", [1, 128], F32, kind="ExternalInput")
    erow1 = nc.dram_tensor("erow1", [1, 128], F32, kind="ExternalInput")
    ycol4 = nc.dram_tensor("ycol4", [128, 4], F32, kind="ExternalInput")
    xrep = nc.dram_tensor("xrep", [128, W], BF16, kind="ExternalInput")
    u = nc.dram_tensor("u", [BPC, CO, H, W], BF16, kind="ExternalOutput")

    with tile.TileContext(nc) as tc:
        with (
            tc.tile_pool(name="consts", bufs=1) as consts,
            tc.tile_pool(name="inp", bufs=1) as in_pool,
            tc.tile_pool(name="outp", bufs=3) as out_pool,
            tc.tile_pool(name="scr", bufs=1) as scratch,
            tc.tile_pool(name="sblkp", bufs=2) as sblk_pool,
            tc.tile_pool(name="s2p", bufs=1) as s2_pool,
            tc.tile_pool(name="bc", bufs=2) as bc_pool,
            tc.tile_pool(name="psumP", bufs=3, space="PSUM") as psum_p,
            tc.tile_pool(name="psumT", bufs=1, space="PSUM") as psum_t,
            tc.tile_pool(name="psumBC", bufs=1, space="PSUM") as psum_bc,
            tc.tile_pool(name="dram", bufs=2, space="DRAM") as dram_pool,
        ):
            sb_y4cat = consts.tile([128, 4], BF16)
            nc.scalar.dma_start(sb_y4cat[:], y4cat[:])
            sb_wty = consts.tile([128, 4, W], BF16)
            nc.scalar.dma_start(sb_wty[:], wty[:])
            sb_wtx = consts.tile([128, 4, W], BF16)
            nc.scalar.dma_start(sb_wtx[:], wtx[:])
            sb_my = consts.tile([128, 2 * CO], F32)
            nc.scalar.dma_start(sb_my[:], my_cat[:])
            sb_mx = consts.tile([128, 2 * CO], F32)
            nc.scalar.dma_start(sb_mx[:], mx_cat[:])
            sb_erow0 = consts.tile([1, 128], F32)
            nc.scalar.dma_start(sb_erow0[:], erow0[:])
            sb_erow1 = consts.tile([1, 128], F32)
            nc.scalar.dma_start(sb_erow1[:], erow1[:])
            sb_ycol4 = consts.tile([128, 4], F32)
            nc.scalar.dma_start(sb_ycol4[:], ycol4[:])
            sb_xrep = consts.tile([128, W], BF16)
            nc.scalar.dma_start(sb_xrep[:], xrep[:])

            # prefetch ALL input tiles (both batches) on the sync queue so
            # the read stream never stalls behind anything else
            in_tiles = []
            for b in range(BPC):
                row = []
                for blk in range(NBLK):
                    t = in_pool.tile(
                        [128, 4, 4, W], BF16, tag=f"in{(b * NBLK + blk) % 12}"
                    )
                    nc.sync.dma_start(
                        t[:],
                        v[b, blk * 8 : blk * 8 + 8, :, :].rearrange(
                            "(i2 ic) (q hh) w -> (ic q) i2 hh w", i2=4, ic=2, q=64, hh=4
                        ),
                    )
                    row.append(t)
                in_tiles.append(row)

            def reduce_phase(b, interleave=None):
                """v[b] tiles -> s2[b]: [128=(ihi r), (hh w)] partial sums."""
                s2 = s2_pool.tile([128, 4, W], BF16, tag=f"s2{b}")
                dscr = dram_pool.tile([32, 4, 2, 512], BF16, tag="dscr")
                drain = 0
                inter = interleave() if interleave is not None else None
                for blk in range(NBLK):
                    if inter is not None:
                        next(inter, None)
                    t = in_tiles[b][blk]
                    s_blk = sblk_pool.tile([4, 4, 2, 512], BF16, tag="sblk")
                    for i2 in range(4):
                        p = psum_p.tile([4, 2, 512], F32, tag="P")
                        for s in range(2):
                            nc.tensor.matmul(
                                p[:, s, :],
                                lhsT=sb_y4cat[:],
                                rhs=t[:, i2, 2 * s : 2 * s + 2, :],
                                start=True,
                                stop=True,
                            )
                        dst = s_blk[:, i2, :, :]
                        if drain % 2 == 0:
                            nc.vector.tensor_copy(dst, p[:])
                        else:
                            nc.scalar.copy(dst, p[:])
                        drain += 1
                    # bounce out: dscr[4*blk + i2, r, s, f] = s_blk[r, i2, s, f]
                    nc.scalar.dma_start(
                        dscr[4 * blk : 4 * blk + 4].rearrange(
                            "i2 r s f -> r i2 s f"
                        ),
                        s_blk[:],
                    )
                # re-partition on readback: dscr[ihi, r, s, f] -> s2[4*ihi + r, :]
                nc.scalar.dma_start(
                    s2[:].rearrange("p hh w -> p (hh w)"),
                    dscr[:].rearrange("ihi r s f -> (ihi r) (s f)"),
                )
                # fused full-width weighted reductions -> gy, gx [128, 1]
                prod = scratch.tile([128, 4, W], BF16, tag="prod")
                gy = scratch.tile([128, 1], F32, tag=f"gy{b}")
                gx = scratch.tile([128, 1], F32, tag=f"gx{b}")
                nc.vector.scalar_tensor_tensor(
                    out=prod[:], in0=s2[:], scalar=1.0, in1=sb_wty[:],
                    op0=MULT, op1=MULT, accum_out=gy[:],
                )
                prod2 = scratch.tile([128, 4, W], BF16, tag="prod")
                nc.vector.scalar_tensor_tensor(
                    out=prod2[:], in0=s2[:], scalar=1.0, in1=sb_wtx[:],
                    op0=MULT, op1=MULT, accum_out=gx[:],
                )
                return gy, gx

            def tiny(b, gy, gx):
                """gy/gx -> ABcols [128, 32, 2] (A,B per ch-pair) + biasT [128, 4, 32]."""
                ab_ps = psum_t.tile([1, 2 * CO], F32, tag="tiny")
                nc.tensor.matmul(
                    ab_ps[:], lhsT=gy[:], rhs=sb_my[:], start=True, stop=False
                )
                nc.tensor.matmul(
                    ab_ps[:], lhsT=gx[:], rhs=sb_mx[:], start=False, stop=True
                )
                ab_row = scratch.tile([1, 2 * CO], F32, tag="ti3")
                nc.vector.tensor_copy(ab_row[:], ab_ps[:])

                # ABcols[p, t, j] = (A,B)[2t + p//64, j]: two rank-1 outer
                # products (indicator-row x ab-row) accumulated on PE
                e_ps = psum_bc.tile([128, 32, 2], F32, tag="bc")
                nc.tensor.matmul(
                    e_ps[:], lhsT=sb_erow0[:], rhs=ab_row[:, 0:CO],
                    start=True, stop=False,
                )
                nc.tensor.matmul(
                    e_ps[:], lhsT=sb_erow1[:], rhs=ab_row[:, CO : 2 * CO],
                    start=False, stop=True,
                )
                abcols = bc_pool.tile([128, 32, 2], F32, tag="abcols")
                nc.vector.tensor_copy(abcols[:], e_ps[:])
                biast = bc_pool.tile([128, 4, 32], F32, tag="biast")
                for hh in range(4):
                    nc.vector.tensor_scalar(
                        out=biast[:, hh, :], in0=abcols[:, :, 0],
                        scalar1=sb_ycol4[:, hh : hh + 1], scalar2=None, op0=MULT,
                    )
                return abcols, biast

            def gen_stream(b, abcols, biast):
                eng = 0
                for tg in range(NTG):
                    yield
                    ot = out_pool.tile([128, 4, 4, W], BF16, tag="out")
                    for tl in range(4):
                        ti = 4 * tg + tl
                        sc_ap = abcols[:, ti, 1:2]
                        for hh in range(4):
                            bias_ap = biast[:, hh, ti : ti + 1]
                            dst = ot[:, tl, hh, :]
                            which = _GEN_ENGINES[eng % len(_GEN_ENGINES)]
                            eng += 1
                            if which == "dve":
                                nc.vector.tensor_scalar(
                                    out=dst, in0=sb_xrep[:], scalar1=sc_ap,
                                    scalar2=bias_ap, op0=MULT, op1=ADD,
                                )
                            elif which == "pool":
                                nc.gpsimd.tensor_scalar(
                                    out=dst, in0=sb_xrep[:], scalar1=sc_ap,
                                    scalar2=bias_ap, op0=MULT, op1=ADD,
                                )
                            else:
                                nc.scalar.activation(
                                    dst, sb_xrep[:],
                                    mybir.ActivationFunctionType.Identity,
                                    bias=bias_ap, scale=sc_ap,
                                )
                    nc.scalar.dma_start(
                        u[b, tg * 8 : tg * 8 + 8, :, :].rearrange(
                            "(tl ic) (q hh) w -> (ic q) tl hh w", tl=4, ic=2, q=64, hh=4
                        ),
                        ot[:],
                    )

            gy0, gx0 = reduce_phase(0)
            ab0 = tiny(0, gy0, gx0)
            g0 = gen_stream(0, *ab0)
            gy1, gx1 = reduce_phase(1, interleave=lambda: g0)
            for _ in g0:
                pass
            ab1 = tiny(1, gy1, gx1)
            for _ in gen_stream(1, *ab1):
                pass

    nc.compile()
    return nc


def make_in_maps(v, psi, phi):
    y = np.linspace(-1.0, 1.0, H, dtype=np.float32)
    x = np.linspace(-1.0, 1.0, W, dtype=np.float32)
    dx = np.float32(2.0 / (W - 1))
    dy = np.float32(2.0 / (H - 1))

    q = np.arange(64)
    # stationary for the h-quad reduction: block-diagonal by channel parity
    # cols: [y4*even, 1*even, y4*odd, 1*odd]; partition p = 64*ic + q
    y4cat = np.zeros((128, 4), np.float32)
    y4cat[0:64, 0] = y[4 * q]
    y4cat[0:64, 1] = 1.0
    y4cat[64:128, 2] = y[4 * q]
    y4cat[64:128, 3] = 1.0

    # s2 partition layout: P = 4*ihi + r, r = 2*ic + role, i = 2*ihi + ic
    # role 0 rows hold y4-weighted sums (weight 1); role 1 rows hold per-hh
    # colsums (Vy correction dy*hh; Vx weight x_w)
    P = np.arange(128)
    role = P % 2
    ic = (P % 4) // 2
    i_of_p = 2 * (P // 4) + ic
    wty = np.zeros((128, 4, W), np.float32)
    wtx = np.zeros((128, 4, W), np.float32)
    wty[role == 0, :, :] = 1.0
    wty[role == 1, :, :] = (dy * np.arange(4, dtype=np.float32))[None, :, None]
    wtx[role == 1, :, :] = x[None, None, :]

    # gy/gx -> inner: psi packs indexed by the same P layout, then folded
    # through phi on the host: m{y,x}{c}[P, 2t+j] = sum_r psi_pack[P, r] *
    # phi[2t+c, r, j], so ab_c = gy^T @ my_c + gx^T @ mx_c directly
    psiy_p = (dx * psi[:, i_of_p, 0].T).astype(np.float32)
    psix_p = (dx * psi[:, i_of_p, 1].T).astype(np.float32)
    psix_p[role == 0, :] = 0.0

    # inner -> (A,B) interleaved per channel pair: phip_c[r, 2t+j] for o=2t+c
    t_idx = np.arange(32)
    phip0 = np.zeros((R, CO), np.float32)
    phip1 = np.zeros((R, CO), np.float32)
    for j in range(2):
        phip0[:, 2 * t_idx + j] = phi[2 * t_idx, :, j].T
        phip1[:, 2 * t_idx + j] = phi[2 * t_idx + 1, :, j].T

    erow0 = np.zeros((1, 128), np.float32)
    erow0[0, 0:64] = 1.0
    erow1 = np.zeros((1, 128), np.float32)
    erow1[0, 64:128] = 1.0

    ycol4 = np.empty((128, 4), np.float32)
    for hh in range(4):
        ycol4[0:64, hh] = y[4 * q + hh]
        ycol4[64:128, hh] = y[4 * q + hh]

    shards = np.ascontiguousarray(
        v.reshape(N_CORES, BPC, CI, H, W).astype(BFNP)
    )
    common = {
        "y4cat": y4cat.astype(BFNP),
        "wty": wty.astype(BFNP),
        "wtx": wtx.astype(BFNP),
        "my_cat": np.concatenate([psiy_p @ phip0, psiy_p @ phip1], axis=1),
        "mx_cat": np.concatenate([psix_p @ phip0, psix_p @ phip1], axis=1),
        "erow0": erow0,
        "erow1": erow1,
        "ycol4": ycol4,
        "xrep": np.broadcast_to(x, (128, W)).astype(BFNP).copy(),
    }
    return [{"v": shards[i], **common} for i in range(N_CORES)]


_NC_CACHE = None


def kernel(v, psi, phi):
    global _NC_CACHE
    if _NC_CACHE is None:
        _NC_CACHE = build_nc()
    nc = _NC_CACHE
    in_maps = make_in_maps(
        np.ascontiguousarray(v, dtype=np.float32),
        np.asarray(psi, dtype=np.float32),
        np.asarray(phi, dtype=np.float32),
    )
    res = run_bass_kernel_spmd(nc, in_maps, core_ids=list(range(N_CORES)))
    return np.concatenate(
        [r["u"].astype(np.float32) for r in res.results], axis=0
    )


if __name__ == "__main__":
    build_nc()
    print("build ok")
